# revision 117
# baseline (speedup 1.0000x reference)
"""Trainium2 Bass kernel for block-causal (chunked) multi-head attention.

Computes, for x:[2,2048,1024], Wqkv:[3072,1024], Wout:[1024,1024]:
    qkv = x @ Wqkv.T ; per-head scaled scores; block-causal mask
    (causal OR same 64-chunk == full attention to all chunks <= own chunk);
    softmax; out = attn @ v ; y = out @ Wout.T

Sharding over 8 NeuronCores: data-parallel over batch (2) x tensor-parallel
over heads (16 heads -> 4 per core).  Each core projects q/k/v for its 4
heads, runs attention, and computes a partial output projection against its
256 columns of Wout; the host sums the 4 partials per batch element.

v2 vs v1 (149us -> 130us in the TimelineSim cost model):
  * All matmul operands in bf16 (PE cost model: 1.0 cycles/row at any free
    size, vs fp32r's 4x penalty below 256 free) and all input/output DMA
    halved.  Host casts inputs to bf16; y partials return as bf16 and are
    summed on the host in f32 (rel_l2 ~6e-3 vs the 2e-2 gate).
  * A tiny warm-up matmul at t~0 pins pe_busy_start so every real matmul
    after ~3us runs at the full 2.4 GHz p-state.
  * Tile-0 projections are emitted kb-major: all 8 accumulation chains (4 qk
    + 4 v) run in parallel across psum banks (one chain per 2KB bank — two
    open groups in one bank are illegal), so the PE consumes each
    (wqk[kb], xt0[kb]) DMA pair as it lands instead of stalling on a
    chain-major walk.  xt0[0]/wv issue via the Pool SWDGE pipe, bypassing
    the serial ~625ns/DMA HWDGE; later x tiles are coarse multi-kb DMAs.
  * vh slot order [h0, h2 | h1, h3] with the ones-columns swapped to the
    low half for odd heads, so the softmax normalization needs no
    partition-shifted DVE copy: both heads' numerators land mul-aligned
    with their reciprocals.
  * Filler schedule: attend(0) <- proj(1); attend(1) <- proj(2)+y(0);
    attend(2) <- proj(3); attend(3) <- y(1)+y(2) (the last attend has no
    next-tile projections to hide its exp latency behind).  PSUM-evacuation
    copies are routed per-phase: qk-chain and y(0) filler copies on Act
    (it has slack in attends 0-2, and the DVE queue gates mm_ps slot
    rotation), y(1)/y(2) and v copies on DVE (attend(3) is exp-bound).
  * Split-tail: in the last attend's final head-pair the main ot
    accumulation stops at block nb-2 (block 15 only touches the last 128
    queries), so three of four token slices normalize and project while
    block 15 still runs; block 15 lands in a separate one-bank psum tile,
    evacuated to SBUF by the Act engine (the BIR verifier rejects DVE ops
    with two PSUM operands) and merged by DVE adds for the last slice.
    Normalize is chunked per 128-token slice interleaved with the y
    matmuls; os0-only halves open their psum groups early; copies go out
    full-width on Act, final slice split DVE/Act in parallel.

Scores are computed transposed (S^T[tk, tq]) so that the attention matmul
needs no transposes, and the softmax denominator comes from ones-columns
interleaved in V.  The block-causal mask is realized structurally: masked
key blocks are never computed, and diagonal blocks use rectangular
sub-views (chunk granularity 64) with one small memset for the corner.
"""

import sys

if "/opt/trn_rl_repo" not in sys.path:
    sys.path.insert(0, "/opt/trn_rl_repo")

from collections import deque

import numpy as np

import concourse.bass as bass  # noqa: F401  (registers types)
import concourse.mybir as mybir
import concourse.tile as tile
from concourse import bacc
from concourse.bass_utils import run_bass_kernel_spmd

F32 = mybir.dt.float32
BF16 = mybir.dt.bfloat16
EXP = mybir.ActivationFunctionType.Exp
COPY = mybir.ActivationFunctionType.Copy

B = 2
T = 2048
DIM = 1024
N_HEADS = 16
HD = 64
CHUNK = 64
H_PER_CORE = 4  # 16 heads / (8 cores / 2 batches)
QT = 512  # query tile (free dim of S^T matmuls)
KB = 128  # key block (contraction block of AV matmuls)
N_QT = T // QT  # 4
N_KB = T // KB  # 16
N_DIMB = DIM // 128  # 8 contraction blocks for the projections
SCALE = 1.0 / np.sqrt(HD)

_CACHED_NC = None


def _emit(nc, tc, xT, wqkT, wvT, woT, y):
    po = tc.tile_pool  # shorthand

    with (
        po(name="persist", bufs=1) as pp,
        po(name="s_ps", bufs=2, space="PSUM") as sps,  # [128,1024] score slots
        po(name="mm_ps", bufs=2, space="PSUM") as mmps,  # [128,512] proj/y slots
        po(name="ot_ps", bufs=2, space="PSUM") as otps,  # [128,512] outT slots
        po(name="pbuf", bufs=8) as ppool,  # exp(S^T) tiles
        po(name="osbuf", bufs=2) as ospool,  # assembled normalized outT
        po(name="rbuf", bufs=4) as rpool,  # reciprocal denominators
        po(name="ybuf", bufs=6) as ypool,
    ):
        # ---- persistent SBUF tensors ----
        # xt for column-tile 0: per-kb tiles (fine DMA deps feed the kb-major
        # projection); tiles 1-3: one [128, 8, 512] tile each, single DMA
        xt0 = [pp.tile([128, QT], BF16, tag=f"xt0_{k}", name=f"xt0_{k}") for k in range(N_DIMB)]
        xtc = [
            pp.tile([128, N_DIMB, QT], BF16, tag=f"xtc{c}", name=f"xtc{c}")
            for c in range(1, N_QT)
        ]
        wqk = [pp.tile([128, 512], BF16, tag=f"wqk{k}", name=f"wqk{k}") for k in range(N_DIMB)]

        def wqk_ap(kb):
            return wqk[kb][:]
        wv = pp.tile([128, N_DIMB, 256], BF16, tag="wv", name="wv")
        wo = pp.tile([128, 2, DIM], BF16, tag="wo", name="wo")
        # q/k head-dim-major: partition block hp holds heads (2hp, 2hp+1)
        qt = [
            [pp.tile([128, QT], BF16, tag=f"qt{i}_{c}", name=f"qt{i}_{c}") for c in range(N_QT)]
            for i in range(2)
        ]
        kt = [
            [pp.tile([128, QT], BF16, tag=f"kt{i}_{c}", name=f"kt{i}_{c}") for c in range(N_QT)]
            for i in range(2)
        ]
        # v (token-major) + ones columns, per key block: [128, slot, 128]
        # slot order is [h0, h2, h1, h3]; even-slot layout [v | ones],
        # odd-head layout [ones | v] (so AV's denominator lands where the
        # normalize mul wants it).  slot(head h) = (h%2)*2 + h//2.
        vh = [
            pp.tile([128, H_PER_CORE, 2 * HD], BF16, tag=f"vh{b}", name=f"vh{b}")
            for b in range(N_KB)
        ]

        def xt_ap(kb, ct):
            return xt0[kb][:] if ct == 0 else xtc[ct - 1][:, kb, :]

        # ---- warm-up: pin pe_busy_start at ~0 so the 3us p-state ramp is
        # done before real data arrives.  Reads the preloaded 1.0 const AP,
        # so it has no dependencies at all and fires right after the barrier.
        cap = nc.const_aps.tensor(1.0, [128, 1], BF16)
        wps = mmps.tile([128, 512], F32, tag="mm512", name="warm_ps")
        nc.tensor.matmul(wps[0:1, 0:1], cap, cap, start=True, stop=True)

        # ---- input DMAs (all SP-issued; HWDGE serializes at ~625ns/DMA).
        # Pair order feeds the kb-major tile-0 projection as it lands; wv
        # inserted early (first v matmuls need it ~5us in); the rest after.
        # xt0[0] and wv go through the Pool engine's SWDGE path — a second,
        # parallel DMA issue pipe (HWDGE serializes at ~625ns/DMA)
        nc.gpsimd.dma_start(xt0[0][:], xT[:, 0, 0:QT])
        nc.gpsimd.dma_start(wv[:], wvT[:])
        for kb in range(N_DIMB):
            nc.sync.dma_start(wqk[kb][:], wqkT[:, kb, :])
            if kb > 0:
                nc.sync.dma_start(xt0[kb][:], xT[:, kb, 0:QT])
        for h in range(2):
            nc.sync.dma_start(
                xtc[0][:, h * 4 : (h + 1) * 4, :], xT[:, h * 4 : (h + 1) * 4, QT : 2 * QT]
            )
        for ct in range(2, N_QT):
            cs = slice(ct * QT, (ct + 1) * QT)
            nc.sync.dma_start(xtc[ct - 1][:], xT[:, :, cs])
        nc.sync.dma_start(wo[:], woT[:])

        def vh_fill(tb, src_ps):
            """src_ps: [128, 256] psum = v features for the 4 heads of this
            key block, column order [v_h0 v_h2 v_h1 v_h3]."""
            s4 = src_ps.rearrange("p (s c) -> p s c", s=4)
            nc.vector.memset(vh[tb][:, 0:2, HD : 2 * HD], 1.0)
            nc.vector.memset(vh[tb][:, 2:4, 0:HD], 1.0)
            nc.vector.tensor_copy(vh[tb][:, 0:2, 0:HD], s4[:, 0:2, :])
            nc.vector.tensor_copy(vh[tb][:, 2:4, HD : 2 * HD], s4[:, 2:4, :])

        def proj0():
            """Tile-0 q/k/v projections, kb-major: 8 accumulation chains in
            parallel across 4 psum tiles so each (wqk[kb], xt0[kb]) DMA pair
            is consumed as it arrives."""
            # one accumulation chain per PSUM bank: qk chains in 512-col bank
            # halves of the 2-bank s2 tiles, each v chain in its own 1-bank
            # tile (two concurrent chains in one bank are illegal)
            qk01 = sps.tile([128, 2 * QT], F32, tag="s2", name="p0qk01")
            qk23 = sps.tile([128, 2 * QT], F32, tag="s2", name="p0qk23")
            vps = [
                (mmps if tb < 2 else otps).tile(
                    [128, 256], F32, tag=("mm512" if tb < 2 else "ot"), name=f"p0v{tb}"
                )
                for tb in range(4)
            ]

            def v_mms(kb):
                for tb in range(4):
                    nc.tensor.matmul(
                        vps[tb][:],
                        xt_ap(kb, 0)[:, tb * KB : (tb + 1) * KB],
                        wv[:, kb, :],
                        start=(kb == 0),
                        stop=(kb == N_DIMB - 1),
                    )

            # v matmuls lag the qk stream by 3 kb so the wv DMA (issued 5th)
            # has landed before the first one fires
            for kb in range(N_DIMB):
                for ob in range(4):
                    ps = qk01 if ob < 2 else qk23
                    nc.tensor.matmul(
                        ps[:, (ob % 2) * QT : (ob % 2 + 1) * QT],
                        wqk_ap(kb)[:, ob * 128 : (ob + 1) * 128],
                        xt_ap(kb, 0),
                        start=(kb == 0),
                        stop=(kb == N_DIMB - 1),
                    )
                if kb >= 3:
                    v_mms(kb - 3)
            for kb in range(N_DIMB - 3, N_DIMB):
                v_mms(kb)
            nc.vector.tensor_copy(qt[0][0][:], qk01[:, 0:QT])
            nc.vector.tensor_copy(qt[1][0][:], qk01[:, QT : 2 * QT])
            nc.vector.tensor_copy(kt[0][0][:], qk23[:, 0:QT])
            nc.vector.tensor_copy(kt[1][0][:], qk23[:, QT : 2 * QT])
            for tb in range(4):
                vh_fill(tb, vps[tb][:])

        def qk_chain(tt, ob):  # ob 0,1 -> q pair blocks; 2,3 -> k pair blocks
            ps = mmps.tile([128, 512], F32, tag="mm512", name=f"qk_ps{tt}_{ob}")
            for kb in range(N_DIMB):
                nc.tensor.matmul(
                    ps[:],
                    wqk_ap(kb)[:, ob * 128 : (ob + 1) * 128],
                    xt_ap(kb, tt),
                    start=(kb == 0),
                    stop=(kb == N_DIMB - 1),
                )
            dest = (qt if ob < 2 else kt)[ob % 2][tt]
            nc.scalar.activation(dest[:], ps[:], COPY)

        def v_chain(tb):
            ps = mmps.tile([128, 256], F32, tag="mm512", name=f"v_ps{tb}")
            for kb in range(N_DIMB):
                nc.tensor.matmul(
                    ps[:],
                    xt_ap(kb, tb // 4)[:, (tb % 4) * KB : (tb % 4 + 1) * KB],
                    wv[:, kb, :],
                    start=(kb == 0),
                    stop=(kb == N_DIMB - 1),
                )
            vh_fill(tb, ps[:])

        def proj_pieces(tt):
            for ob in range(4):
                yield lambda ob=ob: qk_chain(tt, ob)
            for tb in range(4 * tt, 4 * tt + 4):
                yield lambda tb=tb: v_chain(tb)

        def attend(tt, os_pair, fillers, tail_fn=None, late=None):
            nb = 4 * (tt + 1)  # allowed key blocks for this query tile
            # split-tail attends finish fillers early so the mm_ps slots are
            # free for the late blocks' ot2 accumulation
            n_steps = 2 * nb - 3 if tail_fn is not None else 2 * nb - 1
            step = 0
            done_fill = 0
            n_fill = len(fillers)

            def fill():
                nonlocal done_fill
                want = (step + 1) * n_fill // n_steps
                while done_fill < want and fillers:
                    fillers.popleft()()
                    done_fill += 1

            for hp in range(2):  # head pair (2hp, 2hp+1)
                ot = [
                    otps.tile([128, QT], F32, tag="ot", name=f"ot{tt}_{hp}_{i}")
                    for i in range(2)
                ]
                # split-tail (last attend, last pair): blocks nb-2, nb-1 only
                # touch queries >= 256, so the main ot accumulation stops at
                # block nb-3 — the first query-half is then FINAL and its
                # normalize + output projection overlap the last two blocks.
                # The late blocks accumulate into two lazy [128,256] mm_ps
                # tiles (own banks), evacuated to SBUF by the Act engine
                # (the BIR verifier rejects DVE ops with two PSUM operands)
                # and merged into the second half's normalize by DVE adds.
                split = tail_fn is not None and hp == 1
                av_stop = nb - 2 if split else nb - 1
                ot2 = []

                def s_mm(b):
                    """S^T for key block b, both heads, into one 2-bank tile."""
                    diag = b - 4 * tt
                    d = diag * 128 if diag >= 0 else 0
                    s = sps.tile([128, 2 * QT], F32, tag="s2", name=f"s{tt}_{hp}_{b}")
                    for i in range(2):
                        rows = slice(i * 64, i * 64 + 64)
                        nc.tensor.matmul(
                            s[:, i * QT + d : (i + 1) * QT],
                            kt[hp][b // 4][rows, (b % 4) * KB : (b % 4 + 1) * KB],
                            qt[hp][tt][rows, d:QT],
                            start=True,
                            stop=True,
                        )
                    p = ppool.tile([128, 2 * QT], BF16, tag="p", name=f"p{tt}_{hp}_{b}")
                    return s, p

                s_tiles = {0: s_mm(0)}
                for b in range(nb):
                    if b + 1 < nb:
                        s_tiles[b + 1] = s_mm(b + 1)
                    diag = b - 4 * tt
                    d = diag * 128 if diag >= 0 else 0
                    s, p = s_tiles.pop(b)
                    if diag < 0:
                        nc.scalar.activation(p[:], s[:], EXP, scale=SCALE)
                    else:
                        # one exp for both heads over cols >= d (all rows),
                        # then zero the masked corner (rows 64-127 of each
                        # head attend only cols >= d+64) AFTER the exp
                        s2 = s[:].rearrange("p (h c) -> p h c", h=2)
                        p2 = p[:].rearrange("p (h c) -> p h c", h=2)
                        nc.scalar.activation(
                            p2[:, :, d:QT], s2[:, :, d:QT], EXP, scale=SCALE
                        )
                        nc.vector.memset(p2[64:128, :, d : d + 64], 0.0)
                    if split and b == nb - 1:
                        ot2.extend(
                            mmps.tile([128, 128], F32, tag="mm512", name=f"ot2_{i}")
                            for i in range(2)
                        )
                    for i in range(2):
                        # slot order [h0, h2, h1, h3]: head 2hp+i -> slot 2i+hp
                        if split and b > av_stop:
                            nc.tensor.matmul(
                                ot2[i][:, d - 384 : 128],
                                vh[b][:, 2 * i + hp, :],
                                p[:, i * QT + d : (i + 1) * QT],
                                start=True,
                                stop=True,
                            )
                        else:
                            nc.tensor.matmul(
                                ot[i][:, d:QT],
                                vh[b][:, 2 * i + hp, :],
                                p[:, i * QT + d : (i + 1) * QT],
                                start=(b == 0),
                                stop=(b == av_stop),
                            )
                    fill()
                    step += 1

                # normalize:
                #   ot[0] (even head) = [num | den]; ot[1] (odd) = [den | num]
                #   R[0:64] = 1/den0, R[64:128] = 1/den1 (shifted unary ok)
                #   os[0:64] = ot[0][0:64]*R[0:64]  (aligned)
                #   os[64:128] = ot[1][64:128]*R[64:128]  (aligned)
                if hp == 1 and late:
                    # PE work for the normalize window; their copies go to
                    # the Act engine (idle once the last exp is done) so the
                    # DVE normalize chain is not delayed
                    for piece in late:
                        piece()
                rb = rpool.tile([128, QT], F32, tag="rb", name=f"rb{tt}_{hp}")
                if tail_fn is not None and hp == 1:
                    # per-128-token normalize chunks, each immediately
                    # followed by that token slice's output projection; the
                    # os0-only (db=0) halves of the first two slices are
                    # opened pre-norm so the PE has work during the first
                    # normalize ops
                    tail_open, tail_close = tail_fn
                    # evacuate the late blocks' partials to SBUF on the Act
                    # engine (emitted first so they beat the ysb copies in
                    # the Act queue)
                    ot2c = [
                        rpool.tile([128, 128], F32, tag=f"ot2c{i}", name=f"ot2c{i}")
                        for i in range(2)
                    ]
                    for i in range(2):
                        nc.scalar.activation(ot2c[i][:], ot2[i][:], COPY)
                    yopen = {t4: tail_open(t4) for t4 in range(2)}
                    for t4 in range(4):
                        cs = slice(t4 * 128, (t4 + 1) * 128)
                        if t4 < 3:
                            # first query-half: ot is final at block nb-3
                            nc.vector.reciprocal(rb[0:64, cs], ot[0][64:128, cs])
                            nc.vector.reciprocal(rb[64:128, cs], ot[1][0:64, cs])
                            nc.vector.tensor_mul(
                                os_pair[hp][0:64, cs], ot[0][0:64, cs], rb[0:64, cs]
                            )
                            nc.vector.tensor_mul(
                                os_pair[hp][64:128, cs],
                                ot[1][64:128, cs],
                                rb[64:128, cs],
                            )
                        else:
                            # second half: merge the late blocks' partials
                            cs2 = slice(0, 128)
                            sc = [
                                rpool.tile(
                                    [128, 128], F32, tag=f"sc{i}", name=f"sc{t4}_{i}"
                                )
                                for i in range(2)
                            ]
                            nc.vector.tensor_add(sc[0][:], ot[0][:, cs], ot2c[0][:, cs2])
                            nc.vector.tensor_add(sc[1][:], ot[1][:, cs], ot2c[1][:, cs2])
                            nc.vector.reciprocal(rb[0:64, cs], sc[0][64:128, :])
                            nc.vector.tensor_mul(
                                os_pair[hp][0:64, cs], sc[0][0:64, :], rb[0:64, cs]
                            )
                            nc.vector.reciprocal(rb[64:128, cs], sc[1][0:64, :])
                            nc.vector.tensor_mul(
                                os_pair[hp][64:128, cs], sc[1][64:128, :], rb[64:128, cs]
                            )
                        tail_close(t4, yopen.pop(t4))
                        if t4 + 2 < 4:
                            yopen[t4 + 2] = tail_open(t4 + 2)
                else:
                    # per-head op pairs: ot[0]'s reads finish after two ops,
                    # releasing its psum slot for the next head-pair's AV
                    nc.vector.reciprocal(rb[0:64, :], ot[0][64:128, :])
                    nc.vector.tensor_mul(
                        os_pair[hp][0:64, :], ot[0][0:64, :], rb[0:64, :]
                    )
                    nc.vector.reciprocal(rb[64:128, :], ot[1][0:64, :])
                    nc.vector.tensor_mul(
                        os_pair[hp][64:128, :], ot[1][64:128, :], rb[64:128, :]
                    )

            while fillers:
                fillers.popleft()()

        def y_pieces(tt, os_pair, act_copy_from=99, act_all=False):
            """Output projection for query tile tt, run as attend fillers.
            Copies on DVE (the Act engine is exp-bound inside attends) except
            pieces >= act_copy_from, meant to run after the last exp."""
            pieces = []
            for t4 in range(4):
                trows = slice(t4 * 128, (t4 + 1) * 128)
                ysb = ypool.tile([128, DIM], BF16, tag="ysb", name=f"ysb{tt}_{t4}")
                for jb in range(2):
                    idx = 2 * t4 + jb

                    def piece(t4=t4, jb=jb, ysb=ysb, trows=trows, idx=idx):
                        yps = mmps.tile(
                            [128, 512], F32, tag="mm512", name=f"y_ps{tt}_{t4}_{jb}"
                        )
                        for db in range(2):
                            nc.tensor.matmul(
                                yps[:],
                                os_pair[db][:, trows],
                                wo[:, db, jb * 512 : (jb + 1) * 512],
                                start=(db == 0),
                                stop=(db == 1),
                            )
                        dest = ysb[:, jb * 512 : (jb + 1) * 512]
                        if act_all or idx >= act_copy_from:
                            nc.scalar.activation(dest, yps[:], COPY)
                        else:
                            nc.vector.tensor_copy(dest, yps[:])
                        if jb == 1:
                            r0 = tt * QT + t4 * 128
                            nc.sync.dma_start(y[r0 : r0 + 128, :], ysb[:])

                    pieces.append(piece)
            return pieces

        def y_tail_open(tt, os_pair, t4):
            """Start the final tile's output projection for one 128-token
            slice: the db=0 (first head-pair, normalized mid-attend) matmuls
            of both jb halves into one 2-bank s_ps tile.  These only need
            os_pair[0], so they can run while the DVE normalizes os_pair[1]."""
            trows = slice(t4 * 128, (t4 + 1) * 128)
            yps = sps.tile([128, 2 * QT], F32, tag="s2", name=f"yt_ps{tt}_{t4}")
            for jb in range(2):
                nc.tensor.matmul(
                    yps[:, jb * 512 : (jb + 1) * 512],
                    os_pair[0][:, trows],
                    wo[:, 0, jb * 512 : (jb + 1) * 512],
                    start=True,
                    stop=False,
                )
            return yps

        def y_tail_close(tt, os_pair, t4, yps):
            """Finish a tail slice: db=1 accumulation, copy out, DMA."""
            trows = slice(t4 * 128, (t4 + 1) * 128)
            ysb = ypool.tile([128, DIM], BF16, tag="ytb", name=f"yt{tt}_{t4}")
            for jb in range(2):
                nc.tensor.matmul(
                    yps[:, jb * 512 : (jb + 1) * 512],
                    os_pair[1][:, trows],
                    wo[:, 1, jb * 512 : (jb + 1) * 512],
                    start=False,
                    stop=True,
                )
            # full-width Act copy + DMA keeps the DVE free for the normalize
            # chain, which paces this tail; the very last slice splits its
            # copy across DVE+Act halves so the final DMA starts sooner
            r0 = tt * QT + t4 * 128
            if t4 < 3:
                nc.scalar.activation(ysb[:], yps[:], COPY)
                nc.sync.dma_start(y[r0 : r0 + 128, :], ysb[:])
            else:
                # separate half tiles: tile-granular WAW tracking would
                # otherwise serialize the two engines' copies
                y2a = ypool.tile([128, 512], BF16, tag="ytb2a", name=f"yt2a{tt}")
                y2b = ypool.tile([128, 512], BF16, tag="ytb2b", name=f"yt2b{tt}")
                # both halves on the DVE: it is free once the normalize
                # ends, while the Act engine is still draining the previous
                # slice's full-row copy
                nc.vector.tensor_copy(y2a[:], yps[:, 0:512])
                nc.sync.dma_start(y[r0 : r0 + 128, 0:512], y2a[:])
                nc.vector.tensor_copy(y2b[:], yps[:, 512:1024])
                nc.sync.dma_start(y[r0 : r0 + 128, 512:1024], y2b[:])

        # ---- the pipeline ----
        # attend(0) <- proj(1); attend(1) <- proj(2)+y(0); attend(2) <-
        # proj(3); attend(3) <- y(1)+y(2) (reserved: the last attend has no
        # next-tile projections to hide its exp latency behind); y(3) tail.
        def interleave(a, b):
            out = deque()
            a, b = deque(a), deque(b)
            while a or b:
                if a:
                    out.append(a.popleft())
                if b:
                    out.append(b.popleft())
            return out

        proj0()
        os_all = []
        for tt in range(N_QT):
            os_all.append(
                [
                    ospool.tile([128, QT], BF16, tag=f"os{i}_{tt}", name=f"os{i}_{tt}")
                    for i in range(2)
                ]
            )

        attend(0, os_all[0], deque(proj_pieces(1)))
        y0 = y_pieces(0, os_all[0], act_all=True)
        attend(1, os_all[1], interleave(proj_pieces(2), y0))
        y1 = y_pieces(1, os_all[1])
        attend(2, os_all[2], deque(proj_pieces(3)))
        y2 = y_pieces(2, os_all[2])
        attend(
            3,
            os_all[3],
            deque(y1 + y2),
            tail_fn=(
                lambda t4: y_tail_open(3, os_all[3], t4),
                lambda t4, yps: y_tail_close(3, os_all[3], t4, yps),
            ),
        )


def build():
    global _CACHED_NC
    if _CACHED_NC is not None:
        return _CACHED_NC
    nc = bacc.Bacc(
        "TRN2", target_bir_lowering=False, debug=False, enable_asserts=False
    )
    xT = nc.dram_tensor("xT", [128, N_DIMB, T], BF16, kind="ExternalInput").ap()
    wqkT = nc.dram_tensor("wqkT", [128, N_DIMB, 512], BF16, kind="ExternalInput").ap()
    wvT = nc.dram_tensor("wvT", [128, N_DIMB, 256], BF16, kind="ExternalInput").ap()
    woT = nc.dram_tensor("woutT", [128, 2, DIM], BF16, kind="ExternalInput").ap()
    y = nc.dram_tensor("y", [T, DIM], BF16, kind="ExternalOutput").ap()
    with tile.TileContext(nc) as tc:
        _emit(nc, tc, xT, wqkT, wvT, woT, y)
    nc.compile()
    _CACHED_NC = nc
    return nc


def _to_bf16_3d(mat2d, inner):
    """[R, C] f32 -> [128, R//128, C] bf16 with row index (kb*128+p) -> [p, kb]."""
    import ml_dtypes

    r, c = mat2d.shape
    assert r % 128 == 0 and c == inner
    return np.ascontiguousarray(
        mat2d.reshape(r // 128, 128, c).transpose(1, 0, 2)
    ).astype(ml_dtypes.bfloat16)


def make_in_maps(x, Wqkv, Wout):
    """Host-side sharding: core c = (batch c//4, head-group c%4)."""
    in_maps = []
    for c in range(8):
        b, hg = divmod(c, 4)
        hs = hg * H_PER_CORE
        r0, r1 = hs * HD, (hs + H_PER_CORE) * HD
        qrows = Wqkv[r0:r1]
        krows = Wqkv[DIM + r0 : DIM + r1]
        vrows = Wqkv[2 * DIM + r0 : 2 * DIM + r1]
        # v head blocks reordered [h0, h2, h1, h3] to match the vh slot order
        vperm = np.concatenate(
            [vrows[0:64], vrows[128:192], vrows[64:128], vrows[192:256]], 0
        )
        in_maps.append(
            {
                "xT": _to_bf16_3d(np.ascontiguousarray(x[b].T), T),
                "wqkT": _to_bf16_3d(
                    np.ascontiguousarray(np.concatenate([qrows, krows], 0).T), 512
                ),
                "wvT": _to_bf16_3d(np.ascontiguousarray(vperm.T), 256),
                "woutT": _to_bf16_3d(np.ascontiguousarray(Wout[:, r0:r1].T), DIM),
            }
        )
    return in_maps


def kernel(x, Wqkv, Wout):
    x = np.asarray(x, dtype=np.float32)
    Wqkv = np.asarray(Wqkv, dtype=np.float32)
    Wout = np.asarray(Wout, dtype=np.float32)
    nc = build()
    in_maps = make_in_maps(x, Wqkv, Wout)
    res = run_bass_kernel_spmd(nc, in_maps, core_ids=list(range(8)))
    out = np.zeros((B, T, DIM), np.float32)
    for c in range(8):
        out[c // 4] += res.results[c]["y"].astype(np.float32)
    return out
